# revision 4
# baseline (speedup 1.0000x reference)
"""DGM-net forward kernel for Trainium2, 8-core data parallel.

Network (per batch row x of width 101, n_nodes=512, 3 layers):
    S = tanh(x @ W0 + b0)
    for i in 0..2:
        Z = tanh(x @ Uz[i] + S @ Wz[i] + bz[i])
        G = tanh(x @ Ug[i] + S @ Wg[i] + bg[i])
        R = tanh(x @ Ur[i] + S @ Wr[i] + br[i])
        H = tanh(x @ Uh[i] + (S*R) @ Wh[i] + bh[i])
        S = (1-G)*H + Z*S
    out = S @ Wf + bf

Layout strategy: activations are kept feature-major (transposed:
[feature partitions, batch free]) so every matmul uses the weight matrix
in its NATURAL layout as the stationary lhsT operand and the activation
as the moving rhs: out^T[n,b] = sum_k W[k,n] S^T[k,b].  The only
transpose in the whole chain is X -> X^T once per batch chunk, done on
the PE via an identity matmul.

Batch is tiled into chunks of 512 (= one PSUM bank in fp32 and the fp32
moving-operand max).  All weights stay resident in SBUF; per chunk the
whole 3-layer network runs fused without touching DRAM except the X load
and the [1 x 512] output store.

Matmuls run as float32r (PE relaxed-precision fp32: 1 cycle/row vs plain
fp32's 4).  fp32r operands must be *produced* as fp32r, so weight DRAM
params are declared float32r (DMA passthrough) and activation producers
(tanh / DVE updates) write float32r directly.  Accumulation is fp32.
"""
import numpy as np
from contextlib import ExitStack

import concourse.bacc as bacc
import concourse.mybir as mybir
import concourse.tile as tile
from concourse.bass_utils import run_bass_kernel_spmd
from concourse.masks import make_identity

N_CORES = 8
B_FULL = 65536
B = B_FULL // N_CORES      # rows per core
D = 101                    # input width
N = 512                    # n_nodes
L = 3                      # layers
BT = 512                   # batch chunk (free dim of matmuls)
NT = N // 128              # output-feature tiles per gate
KT = N // 128              # contraction tiles for S@W
FP = mybir.dt.float32
FR = mybir.dt.float32r

GATES = ("z", "g", "r", "h")


def _build(mm_dt=FR):
    nc = bacc.Bacc(None)
    Tanh = mybir.ActivationFunctionType.Tanh
    WDT = mm_dt                     # weight / matmul-operand dtype

    X = nc.declare_dram_parameter("X", [B, D], FP, isOutput=False)
    W0d = nc.declare_dram_parameter("W0", [D, N], WDT, isOutput=False)
    b0d = nc.declare_dram_parameter("b0", [1, N], FP, isOutput=False)
    Ud = {g: nc.declare_dram_parameter(f"U{g}", [L, D, N], WDT, isOutput=False)
          for g in GATES}
    Wd = {g: nc.declare_dram_parameter(f"W{g}", [L, N, N], WDT, isOutput=False)
          for g in GATES}
    bd = {g: nc.declare_dram_parameter(f"b{g}", [L, 1, N], FP, isOutput=False)
          for g in GATES}
    Wfd = nc.declare_dram_parameter("Wf", [N, 1], WDT, isOutput=False)
    bfd = nc.declare_dram_parameter("bf", [1, 1], FP, isOutput=False)
    OUT = nc.declare_dram_parameter("out", [B, 1], FP, isOutput=True)

    with tile.TileContext(nc) as tc, ExitStack() as ctx:
        consts = ctx.enter_context(tc.tile_pool(name="consts", bufs=1))
        xpool = ctx.enter_context(tc.tile_pool(name="x", bufs=4))
        xtpool = ctx.enter_context(tc.tile_pool(name="xt", bufs=2))
        spool = ctx.enter_context(tc.tile_pool(name="s", bufs=1))
        zpool = ctx.enter_context(tc.tile_pool(name="z", bufs=1))
        gpool = ctx.enter_context(tc.tile_pool(name="g", bufs=1))
        rpool = ctx.enter_context(tc.tile_pool(name="r", bufs=1))
        hpool = ctx.enter_context(tc.tile_pool(name="h", bufs=1))
        opool = ctx.enter_context(tc.tile_pool(name="o", bufs=2))
        psum = ctx.enter_context(tc.tile_pool(name="psum", bufs=4, space="PSUM"))
        psum_t = ctx.enter_context(tc.tile_pool(name="psum_t", bufs=2, space="PSUM"))
        psum_f = ctx.enter_context(tc.tile_pool(name="psum_f", bufs=2, space="PSUM"))

        ident = consts.tile([128, 128], FP)
        make_identity(nc, ident[:])

        # --- resident weights, all in natural (k-major) layout ---
        w0 = consts.tile([D, N], WDT)
        nc.sync.dma_start(out=w0[:], in_=W0d[:])
        b0t = consts.tile([128, NT], FP)
        nc.sync.dma_start(out=b0t[:],
                          in_=b0d[:].rearrange("o (nt p) -> p (o nt)", p=128))
        u, w, bias = {}, {}, {}
        for g in GATES:
            u[g] = consts.tile([D, L, N], WDT, name=f"u_{g}")
            nc.sync.dma_start(out=u[g][:], in_=Ud[g][:].rearrange("l p n -> p l n"))
            w[g] = consts.tile([128, L, KT, N], WDT, name=f"w_{g}")
            nc.sync.dma_start(
                out=w[g][:],
                in_=Wd[g][:].rearrange("l (kt p) n -> p l kt n", p=128))
            bias[g] = consts.tile([128, L * NT], FP, name=f"bias_{g}")
            nc.sync.dma_start(
                out=bias[g][:],
                in_=bd[g][:].rearrange("l o (nt p) -> p (l o nt)", p=128))
        wf = consts.tile([128, KT], WDT)
        nc.sync.dma_start(out=wf[:], in_=Wfd[:].rearrange("(kt p) o -> p (kt o)", p=128))
        bft = consts.tile([1, 1], FP)
        nc.sync.dma_start(out=bft[:], in_=bfd[:])

        sub, mult = mybir.AluOpType.subtract, mybir.AluOpType.mult

        def f32(ap):            # read fp32r bits as plain fp32 (DVE/ACT reads)
            return ap.bitcast(FP) if ap.dtype != FP else ap

        for c in range(B // BT):
            r0 = c * BT
            # ---- X^T chunk via PE transpose; rounded to fp32r on copy-out ----
            xt = xtpool.tile([D, BT], WDT)
            for i in range(BT // 128):
                xrow = xpool.tile([128, D], FP)
                nc.sync.dma_start(out=xrow[:],
                                  in_=X[r0 + i * 128: r0 + (i + 1) * 128, :])
                tp = psum_t.tile([D, 128], FP)
                nc.tensor.transpose(tp[:], xrow[:], ident[:])
                nc.vector.tensor_copy(xt[:, i * 128:(i + 1) * 128], tp[:])

            # ---- S0 = tanh(X @ W0 + b0) ----
            s = spool.tile([128, KT, BT], WDT)
            for nt in range(NT):
                acc = psum.tile([128, BT], FP)
                nc.tensor.matmul(acc[:], w0[:, nt * 128:(nt + 1) * 128],
                                 xt[:], start=True, stop=True)
                nc.scalar.activation(s[:, nt, :], acc[:], Tanh,
                                     bias=b0t[:, nt:nt + 1])

            for l in range(L):
                zt = zpool.tile([128, NT, BT], FP)
                gt = gpool.tile([128, NT, BT], FP)
                rt = rpool.tile([128, NT, BT], WDT)
                ht = hpool.tile([128, NT, BT], FP)
                # Z, G, R gates: tanh(X@U + S@W + b)
                for g, dest in (("z", zt), ("g", gt), ("r", rt)):
                    for nt in range(NT):
                        acc = psum.tile([128, BT], FP)
                        nc.tensor.matmul(
                            acc[:], u[g][:, l, nt * 128:(nt + 1) * 128],
                            xt[:], start=True, stop=False)
                        for kt in range(KT):
                            nc.tensor.matmul(
                                acc[:],
                                w[g][:, l, kt, nt * 128:(nt + 1) * 128],
                                s[:, kt, :], start=False, stop=(kt == KT - 1))
                        nc.scalar.activation(dest[:, nt, :], acc[:], Tanh,
                                             bias=bias[g][:, l * NT + nt:l * NT + nt + 1])
                # R <- S*R (in place, rounded to fp32r; R only feeds (S*R)@Wh)
                for kt in range(KT):
                    nc.vector.tensor_mul(rt[:, kt, :], f32(s[:, kt, :]),
                                         f32(rt[:, kt, :]))
                # H = tanh(X@Uh + (S*R)@Wh + bh)
                for nt in range(NT):
                    acc = psum.tile([128, BT], FP)
                    nc.tensor.matmul(
                        acc[:], u["h"][:, l, nt * 128:(nt + 1) * 128],
                        xt[:], start=True, stop=False)
                    for kt in range(KT):
                        nc.tensor.matmul(
                            acc[:], w["h"][:, l, kt, nt * 128:(nt + 1) * 128],
                            rt[:, kt, :], start=False, stop=(kt == KT - 1))
                    nc.scalar.activation(ht[:, nt, :], acc[:], Tanh,
                                         bias=bias["h"][:, l * NT + nt:l * NT + nt + 1])
                # S = Z*S + (1-G)*H   (3 DVE ops per feature tile, in place)
                for nt in range(NT):
                    nc.vector.tensor_mul(zt[:, nt, :], zt[:, nt, :],
                                         f32(s[:, nt, :]))
                    nc.vector.scalar_tensor_tensor(
                        gt[:, nt, :], gt[:, nt, :], 1.0, ht[:, nt, :],
                        op0=sub, op1=mult)          # (G-1)*H
                    nc.vector.tensor_sub(s[:, nt, :], zt[:, nt, :], gt[:, nt, :])

            # ---- out = S @ Wf + bf ----
            accf = psum_f.tile([1, BT], FP)
            for kt in range(KT):
                nc.tensor.matmul(accf[:], wf[:, kt:kt + 1], s[:, kt, :],
                                 start=(kt == 0), stop=(kt == KT - 1))
            ot = opool.tile([1, BT], FP)
            nc.vector.tensor_scalar_add(ot[:], accf[:], bft[0:1, 0:1])
            nc.sync.dma_start(out=OUT[r0:r0 + BT, 0:1].rearrange("b o -> o b"),
                              in_=ot[:])

    nc.compile()
    return nc


_NC = None


def _get_nc():
    global _NC
    if _NC is None:
        _NC = _build()
    return _NC


def _run(inputs, **kw):
    nc = _get_nc()
    names = (["W0", "b0"] + [f"U{g}" for g in GATES] + [f"W{g}" for g in GATES]
             + [f"b{g}" for g in GATES] + ["Wf", "bf"])
    shared = {n: np.ascontiguousarray(np.asarray(inputs[n], np.float32))
              for n in names}
    X = np.asarray(inputs["X"], np.float32)
    in_maps = [dict(shared, X=np.ascontiguousarray(X[i * B:(i + 1) * B]))
               for i in range(N_CORES)]
    res = run_bass_kernel_spmd(nc, in_maps, list(range(N_CORES)), **kw)
    out = np.concatenate([res.results[i]["out"] for i in range(N_CORES)], axis=0)
    return out, res


def kernel(**inputs) -> np.ndarray:
    out, _ = _run(inputs)
    return out


# revision 7
# speedup vs baseline: 14.4469x; 14.4469x over previous
"""DGM-net forward kernel for Trainium2, 8-core data parallel.

Network (per batch row x of width 101, n_nodes=512, 3 layers):
    S = tanh(x @ W0 + b0)
    for i in 0..2:
        Z = tanh(x @ Uz[i] + S @ Wz[i] + bz[i])
        G = tanh(x @ Ug[i] + S @ Wg[i] + bg[i])
        R = tanh(x @ Ur[i] + S @ Wr[i] + br[i])
        H = tanh(x @ Uh[i] + (S*R) @ Wh[i] + bh[i])
        S = (1-G)*H + Z*S
    out = S @ Wf + bf

Layout strategy: activations are kept feature-major (transposed:
[feature partitions, batch free]) so every matmul uses the weight matrix
in its NATURAL layout as the stationary lhsT operand and the activation
as the moving rhs: out^T[n,b] = sum_k W[k,n] S^T[k,b].  The only
transpose in the whole chain is X -> X^T once per batch chunk, done on
the PE via an identity matmul.

Batch is tiled into chunks of 512 (= one PSUM bank in fp32 and the fp32
moving-operand max).  All weights stay resident in SBUF; per chunk the
whole 3-layer network runs fused without touching DRAM except the X load
and the [1 x 512] output store.

Matmuls run as float32r (PE relaxed-precision fp32: 1 cycle/row vs plain
fp32's 4).  fp32r operands must be *produced* as fp32r, so weight DRAM
params are declared float32r (DMA passthrough) and activation producers
(tanh / DVE updates) write float32r directly.  Accumulation is fp32.
"""
import numpy as np
from contextlib import ExitStack

import concourse.bacc as bacc
import concourse.mybir as mybir
import concourse.tile as tile
from concourse.bass_utils import run_bass_kernel_spmd
from concourse.masks import make_identity

N_CORES = 8
B_FULL = 65536
B = B_FULL // N_CORES      # rows per core
D = 101                    # input width
N = 512                    # n_nodes
L = 3                      # layers
BT = 512                   # batch chunk (free dim of matmuls)
NT = N // 128              # output-feature tiles per gate
KT = N // 128              # contraction tiles for S@W
FP = mybir.dt.float32
FR = mybir.dt.float32r

GATES = ("z", "g", "r", "h")


def _build(mm_dt=FR, reps=1):
    nc = bacc.Bacc(None)
    Tanh = mybir.ActivationFunctionType.Tanh
    WDT = mm_dt                     # weight / matmul-operand dtype

    X = nc.declare_dram_parameter("X", [B, D], FP, isOutput=False)
    W0d = nc.declare_dram_parameter("W0", [D, N], WDT, isOutput=False)
    b0d = nc.declare_dram_parameter("b0", [1, N], FP, isOutput=False)
    Ud = {g: nc.declare_dram_parameter(f"U{g}", [L, D, N], WDT, isOutput=False)
          for g in GATES}
    Wd = {g: nc.declare_dram_parameter(f"W{g}", [L, N, N], WDT, isOutput=False)
          for g in GATES}
    bd = {g: nc.declare_dram_parameter(f"b{g}", [L, 1, N], FP, isOutput=False)
          for g in GATES}
    Wfd = nc.declare_dram_parameter("Wf", [N, 1], WDT, isOutput=False)
    bfd = nc.declare_dram_parameter("bf", [1, 1], FP, isOutput=False)
    OUT = nc.declare_dram_parameter("out", [B, 1], FP, isOutput=True)

    with tile.TileContext(nc) as tc, ExitStack() as ctx:
        consts = ctx.enter_context(tc.tile_pool(name="consts", bufs=1))
        xpool = ctx.enter_context(tc.tile_pool(name="x", bufs=4))
        xtpool = ctx.enter_context(tc.tile_pool(name="xt", bufs=2))
        spool = ctx.enter_context(tc.tile_pool(name="s", bufs=1))
        zpool = ctx.enter_context(tc.tile_pool(name="z", bufs=1))
        gpool = ctx.enter_context(tc.tile_pool(name="g", bufs=1))
        rpool = ctx.enter_context(tc.tile_pool(name="r", bufs=1))
        hpool = ctx.enter_context(tc.tile_pool(name="h", bufs=1))
        opool = ctx.enter_context(tc.tile_pool(name="o", bufs=2))
        psum = ctx.enter_context(tc.tile_pool(name="psum", bufs=4, space="PSUM"))
        psum_t = ctx.enter_context(tc.tile_pool(name="psum_t", bufs=2, space="PSUM"))
        psum_f = ctx.enter_context(tc.tile_pool(name="psum_f", bufs=2, space="PSUM"))

        ident = consts.tile([128, 128], FP)
        make_identity(nc, ident[:])

        # --- resident weights, all in natural (k-major) layout ---
        w0 = consts.tile([D, N], WDT)
        nc.sync.dma_start(out=w0[:], in_=W0d[:])
        b0t = consts.tile([128, NT], FP)
        nc.sync.dma_start(out=b0t[:],
                          in_=b0d[:].rearrange("o (nt p) -> p (o nt)", p=128))
        u, w, bias = {}, {}, {}
        for g in GATES:
            u[g] = consts.tile([D, L, N], WDT, name=f"u_{g}")
            nc.sync.dma_start(out=u[g][:], in_=Ud[g][:].rearrange("l p n -> p l n"))
            w[g] = consts.tile([128, L, KT, N], WDT, name=f"w_{g}")
            nc.sync.dma_start(
                out=w[g][:],
                in_=Wd[g][:].rearrange("l (kt p) n -> p l kt n", p=128))
            bias[g] = consts.tile([128, L * NT], FP, name=f"bias_{g}")
            nc.sync.dma_start(
                out=bias[g][:],
                in_=bd[g][:].rearrange("l o (nt p) -> p (l o nt)", p=128))
        wf = consts.tile([128, KT], WDT)
        nc.sync.dma_start(out=wf[:], in_=Wfd[:].rearrange("(kt p) o -> p (kt o)", p=128))
        bft = consts.tile([1, 1], FP)
        nc.sync.dma_start(out=bft[:], in_=bfd[:])

        sub, mult = mybir.AluOpType.subtract, mybir.AluOpType.mult

        def f32(ap):            # read fp32r bits as plain fp32 (DVE/ACT reads)
            return ap.bitcast(FP) if ap.dtype != FP else ap

        def emit_chunk(c):
            r0 = c * BT
            # ---- X^T chunk via PE transpose; rounded to fp32r on copy-out ----
            xt = xtpool.tile([D, BT], WDT)
            for i in range(BT // 128):
                xrow = xpool.tile([128, D], FP)
                nc.sync.dma_start(out=xrow[:],
                                  in_=X[r0 + i * 128: r0 + (i + 1) * 128, :])
                tp = psum_t.tile([D, 128], FP)
                nc.tensor.transpose(tp[:], xrow[:], ident[:])
                nc.vector.tensor_copy(xt[:, i * 128:(i + 1) * 128], tp[:])

            # ---- S0 = tanh(X @ W0 + b0) ----
            s = spool.tile([128, KT, BT], WDT)
            for nt in range(NT):
                acc = psum.tile([128, BT], FP)
                nc.tensor.matmul(acc[:], w0[:, nt * 128:(nt + 1) * 128],
                                 xt[:], start=True, stop=True)
                nc.scalar.activation(s[:, nt, :], acc[:], Tanh,
                                     bias=b0t[:, nt:nt + 1])

            for l in range(L):
                zt = zpool.tile([128, NT, BT], FP)
                gt = gpool.tile([128, NT, BT], FP)
                rt = rpool.tile([128, NT, BT], WDT)
                ht = hpool.tile([128, NT, BT], FP)
                # Z, G, R gates: tanh(X@U + S@W + b)
                for g, dest in (("z", zt), ("g", gt), ("r", rt)):
                    for nt in range(NT):
                        acc = psum.tile([128, BT], FP)
                        nc.tensor.matmul(
                            acc[:], u[g][:, l, nt * 128:(nt + 1) * 128],
                            xt[:], start=True, stop=False)
                        for kt in range(KT):
                            nc.tensor.matmul(
                                acc[:],
                                w[g][:, l, kt, nt * 128:(nt + 1) * 128],
                                s[:, kt, :], start=False, stop=(kt == KT - 1))
                        nc.scalar.activation(dest[:, nt, :], acc[:], Tanh,
                                             bias=bias[g][:, l * NT + nt:l * NT + nt + 1])
                # R <- S*R (in place, rounded to fp32r; R only feeds (S*R)@Wh)
                for kt in range(KT):
                    nc.vector.tensor_mul(rt[:, kt, :], f32(s[:, kt, :]),
                                         f32(rt[:, kt, :]))
                # H = tanh(X@Uh + (S*R)@Wh + bh)
                for nt in range(NT):
                    acc = psum.tile([128, BT], FP)
                    nc.tensor.matmul(
                        acc[:], u["h"][:, l, nt * 128:(nt + 1) * 128],
                        xt[:], start=True, stop=False)
                    for kt in range(KT):
                        nc.tensor.matmul(
                            acc[:], w["h"][:, l, kt, nt * 128:(nt + 1) * 128],
                            rt[:, kt, :], start=False, stop=(kt == KT - 1))
                    nc.scalar.activation(ht[:, nt, :], acc[:], Tanh,
                                         bias=bias["h"][:, l * NT + nt:l * NT + nt + 1])
                # S = Z*S + (1-G)*H   (3 DVE ops per feature tile, in place)
                for nt in range(NT):
                    nc.vector.tensor_mul(zt[:, nt, :], zt[:, nt, :],
                                         f32(s[:, nt, :]))
                    nc.vector.scalar_tensor_tensor(
                        gt[:, nt, :], gt[:, nt, :], 1.0, ht[:, nt, :],
                        op0=sub, op1=mult)          # (G-1)*H
                    nc.vector.tensor_sub(s[:, nt, :], zt[:, nt, :], gt[:, nt, :])

            # ---- out = S @ Wf + bf ----
            accf = psum_f.tile([1, BT], FP)
            for kt in range(KT):
                nc.tensor.matmul(accf[:], wf[:, kt:kt + 1], s[:, kt, :],
                                 start=(kt == 0), stop=(kt == KT - 1))
            ot = opool.tile([1, BT], FP)
            nc.vector.tensor_scalar_add(ot[:], accf[:], bft[0:1, 0:1])
            nc.sync.dma_start(out=OUT[r0:r0 + BT, 0:1].rearrange("b o -> o b"),
                              in_=ot[:])

        if reps == 1:
            for c in range(B // BT):
                emit_chunk(c)
        else:           # device-side repetition loop, for benchmarking only
            with tc.For_i(0, reps):
                for c in range(B // BT):
                    emit_chunk(c)

    nc.compile()
    return nc


_NC = None


def _get_nc():
    global _NC
    if _NC is None:
        _NC = _build()
    return _NC


def _run(inputs, **kw):
    nc = _get_nc()
    names = (["W0", "b0"] + [f"U{g}" for g in GATES] + [f"W{g}" for g in GATES]
             + [f"b{g}" for g in GATES] + ["Wf", "bf"])
    shared = {n: np.ascontiguousarray(np.asarray(inputs[n], np.float32))
              for n in names}
    X = np.asarray(inputs["X"], np.float32)
    in_maps = [dict(shared, X=np.ascontiguousarray(X[i * B:(i + 1) * B]))
               for i in range(N_CORES)]
    res = run_bass_kernel_spmd(nc, in_maps, list(range(N_CORES)), **kw)
    out = np.concatenate([res.results[i]["out"] for i in range(N_CORES)], axis=0)
    return out, res


def kernel(**inputs) -> np.ndarray:
    out, _ = _run(inputs)
    return out


# revision 23
# speedup vs baseline: 16.8598x; 1.1670x over previous
"""DGM-net forward kernel for Trainium2, 8-core data parallel.

Network (per batch row x of width 101, n_nodes=512, 3 layers):
    S = tanh(x @ W0 + b0)
    for i in 0..2:
        Z = tanh(x @ Uz[i] + S @ Wz[i] + bz[i])
        G = tanh(x @ Ug[i] + S @ Wg[i] + bg[i])
        R = tanh(x @ Ur[i] + S @ Wr[i] + br[i])
        H = tanh(x @ Uh[i] + (S*R) @ Wh[i] + bh[i])
        S = (1-G)*H + Z*S
    out = S @ Wf + bf

Layout strategy: activations are kept feature-major (transposed:
[feature partitions, batch free]) so every matmul uses the weight matrix
in its NATURAL layout as the stationary lhsT operand and the activation
as the moving rhs: out^T[n,b] = sum_k W[k,n] S^T[k,b].  The only
transpose in the whole chain is X -> X^T once per batch chunk, done on
the PE via an identity matmul.

Batch is tiled into chunks of 512 (= one PSUM bank in fp32 and the fp32
moving-operand max).  All weights stay resident in SBUF; per chunk the
whole 3-layer network runs fused without touching DRAM except the X load
and the [1 x 512] output store.

Matmuls run as float32r (PE relaxed-precision fp32: 1 cycle/row vs plain
fp32's 4).  fp32r operands must be *produced* as fp32r, so weight DRAM
params are declared float32r (DMA passthrough) and activation producers
(tanh / DVE updates) write float32r directly.  Accumulation is fp32.
"""
import numpy as np
from contextlib import ExitStack

import concourse.bacc as bacc
import concourse.mybir as mybir
import concourse.tile as tile
from concourse.bass_utils import run_bass_kernel_spmd
from concourse.masks import make_identity

N_CORES = 8
B_FULL = 65536
B = B_FULL // N_CORES      # rows per core
D = 101                    # input width
N = 512                    # n_nodes
L = 3                      # layers
BT = 512                   # batch chunk (free dim of matmuls)
NT = N // 128              # output-feature tiles per gate
KT = N // 128              # contraction tiles for S@W
FP = mybir.dt.float32
FR = mybir.dt.float32r

GATES = ("z", "g", "r", "h")


def _build(mm_dt=FR, reps=1):
    nc = bacc.Bacc(None)
    Tanh = mybir.ActivationFunctionType.Tanh
    WDT = mm_dt                     # weight / matmul-operand dtype

    X = nc.declare_dram_parameter("X", [B, D], FP, isOutput=False)
    W0d = nc.declare_dram_parameter("W0", [D, N], WDT, isOutput=False)
    b0d = nc.declare_dram_parameter("b0", [1, N], FP, isOutput=False)
    Ud = {g: nc.declare_dram_parameter(f"U{g}", [L, D, N], WDT, isOutput=False)
          for g in GATES}
    Wd = {g: nc.declare_dram_parameter(f"W{g}", [L, N, N], WDT, isOutput=False)
          for g in GATES}
    bd = {g: nc.declare_dram_parameter(f"b{g}", [L, 1, N], FP, isOutput=False)
          for g in GATES}
    Wfd = nc.declare_dram_parameter("Wf", [N, 1], WDT, isOutput=False)
    bfd = nc.declare_dram_parameter("bf", [1, 1], FP, isOutput=False)
    OUT = nc.declare_dram_parameter("out", [B, 1], FP, isOutput=True)

    with tile.TileContext(nc) as tc, ExitStack() as ctx:
        consts = ctx.enter_context(tc.tile_pool(name="consts", bufs=1))
        xpool = ctx.enter_context(tc.tile_pool(name="x", bufs=8))
        xtpool = ctx.enter_context(tc.tile_pool(name="xt", bufs=2))
        spool = ctx.enter_context(tc.tile_pool(name="s", bufs=2))
        zpool = ctx.enter_context(tc.tile_pool(name="z", bufs=1))
        gpool = ctx.enter_context(tc.tile_pool(name="g", bufs=1))
        rpool = ctx.enter_context(tc.tile_pool(name="r", bufs=1))
        hpool = ctx.enter_context(tc.tile_pool(name="h", bufs=1))
        opool = ctx.enter_context(tc.tile_pool(name="o", bufs=2))
        psum = ctx.enter_context(tc.tile_pool(name="psum", bufs=5, space="PSUM"))
        psum_t = ctx.enter_context(tc.tile_pool(name="psum_t", bufs=2, space="PSUM"))
        psum_f = ctx.enter_context(tc.tile_pool(name="psum_f", bufs=1, space="PSUM"))

        ident = consts.tile([128, 128], FP)
        make_identity(nc, ident[:])

        # --- resident weights, all in natural (k-major) layout ---
        # Weight DMAs go through the ACT HWDGE queue (idle at startup; X
        # loads use the sync queue), split per layer and issued in the
        # order the first chunk consumes them so the PE starts ASAP.
        def wdma(out, in_):
            # weights go through GpSimd's SWDGE queue: its sequencer has no
            # compute role, so weight streaming never blocks ACT/SP streams
            nc.gpsimd.dma_start(out=out, in_=in_)

        # per-(gate,layer) tiles so dependency tracking and DMA arrival are
        # layer-granular: layer-0 matmuls start as soon as layer-0 weights land
        w0 = consts.tile([D, N], WDT)
        b0t = consts.tile([128, NT], FP)
        u, w, bias = {}, {}, {}
        for g in GATES:
            for l in range(L):
                u[g, l] = consts.tile([D, N], WDT, name=f"u_{g}{l}")
                w[g, l] = consts.tile([128, KT, N], WDT, name=f"w_{g}{l}")
            bias[g] = consts.tile([128, L * NT], FP, name=f"bias_{g}")
        wf = consts.tile([128, KT], WDT)
        bft = consts.tile([1, 1], FP)

        def emit_weight_dmas():
            wdma(w0[:], W0d[:])
            wdma(b0t[:], b0d[:].rearrange("o (nt p) -> p (o nt)", p=128))
            for g in GATES:
                wdma(bias[g][:], bd[g][:].rearrange("l o (nt p) -> p (l o nt)", p=128))
            for l in range(L):
                for g in GATES:
                    wdma(u[g, l][:], Ud[g][l].rearrange("p n -> p n"))
                    # W split in half so both queues stream each matrix
                    wdma(w[g, l][:, 0:2],
                         Wd[g][l, 0:256].rearrange("(kt p) n -> p kt n", p=128))
                    wdma(w[g, l][:, 2:4],
                         Wd[g][l, 256:512].rearrange("(kt p) n -> p kt n", p=128))
            wdma(wf[:], Wfd[:].rearrange("(kt p) o -> p (kt o)", p=128))
            wdma(bft[:], bfd[:])

        sub, mult = mybir.AluOpType.subtract, mybir.AluOpType.mult

        def f32(ap):            # read fp32r bits as plain fp32 (DVE/ACT reads)
            return ap.bitcast(FP) if ap.dtype != FP else ap

        def load_x(c):
            r0 = c * BT
            rows = []
            for i in range(BT // 128):
                xrow = xpool.tile([128, D], FP, name="xrow")
                nc.sync.dma_start(out=xrow[:],
                                  in_=X[r0 + i * 128: r0 + (i + 1) * 128, :])
                rows.append(xrow)
            return rows

        def emit_head(c, rows=None):
            """X load + PE transpose + S0 for chunk c; returns (xt, s)."""
            if rows is None:
                rows = load_x(c)
            xt = xtpool.tile([D, BT], WDT)
            tp = psum_t.tile([D, BT], FP)      # one PSUM bank, 4 transposes
            for i in range(BT // 128):
                nc.tensor.transpose(tp[:, i * 128:(i + 1) * 128], rows[i][:],
                                    ident[:])
            nc.scalar.copy(xt[:], tp[:])

            # ---- S0 = tanh(X @ W0 + b0) ----
            s = spool.tile([128, KT, BT], WDT)
            for nt in range(NT):
                acc = psum.tile([128, BT], FP)
                nc.tensor.matmul(acc[:], w0[:, nt * 128:(nt + 1) * 128],
                                 xt[:], start=True, stop=True)
                nc.scalar.activation(s[:, nt, :], acc[:], Tanh,
                                     bias=b0t[:, nt:nt + 1])
            return xt, s

        def emit_layer(l, xt, s):
            if True:
                zt = zpool.tile([128, NT, BT], FP)
                gt = gpool.tile([128, NT, BT], FP)
                rt = rpool.tile([128, NT, BT], WDT)
                ht = hpool.tile([128, NT, BT], FP)
                # Z, G, R gates: tanh(X@U + S@W + b)
                for g, dest in (("z", zt), ("g", gt), ("r", rt)):
                    for nt in range(NT):
                        acc = psum.tile([128, BT], FP)
                        nc.tensor.matmul(
                            acc[:], u[g, l][:, nt * 128:(nt + 1) * 128],
                            xt[:], start=True, stop=False)
                        for kt in range(KT):
                            nc.tensor.matmul(
                                acc[:],
                                w[g, l][:, kt, nt * 128:(nt + 1) * 128],
                                s[:, kt, :], start=False, stop=(kt == KT - 1))
                        nc.scalar.activation(dest[:, nt, :], acc[:], Tanh,
                                             bias=bias[g][:, l * NT + nt:l * NT + nt + 1])
                # R <- S*R (in place, rounded to fp32r; R only feeds (S*R)@Wh)
                for kt in range(KT):
                    nc.vector.tensor_mul(rt[:, kt, :], f32(s[:, kt, :]),
                                         f32(rt[:, kt, :]))
                # H = tanh(X@Uh + (S*R)@Wh + bh)
                for nt in range(NT):
                    acc = psum.tile([128, BT], FP)
                    nc.tensor.matmul(
                        acc[:], u["h", l][:, nt * 128:(nt + 1) * 128],
                        xt[:], start=True, stop=False)
                    for kt in range(KT):
                        nc.tensor.matmul(
                            acc[:], w["h", l][:, kt, nt * 128:(nt + 1) * 128],
                            rt[:, kt, :], start=False, stop=(kt == KT - 1))
                    nc.scalar.activation(ht[:, nt, :], acc[:], Tanh,
                                         bias=bias["h"][:, l * NT + nt:l * NT + nt + 1])
                # S = Z*S + (1-G)*H   (3 DVE ops per feature tile, in place)
                for nt in range(NT):
                    nc.vector.tensor_mul(zt[:, nt, :], zt[:, nt, :],
                                         f32(s[:, nt, :]))
                    nc.vector.scalar_tensor_tensor(
                        gt[:, nt, :], gt[:, nt, :], 1.0, ht[:, nt, :],
                        op0=sub, op1=mult)          # (G-1)*H
                    nc.vector.tensor_sub(s[:, nt, :], zt[:, nt, :], gt[:, nt, :])

        def emit_final(c, s):
            # ---- out = S @ Wf + bf ----
            r0 = c * BT
            accf = psum_f.tile([1, BT], FP)
            for kt in range(KT):
                nc.tensor.matmul(accf[:], wf[:, kt:kt + 1], s[:, kt, :],
                                 start=(kt == 0), stop=(kt == KT - 1))
            ot = opool.tile([1, BT], FP)
            nc.vector.tensor_scalar_add(ot[:], accf[:], bft[0:1, 0:1])
            nc.sync.dma_start(out=OUT[r0:r0 + BT, 0:1].rearrange("b o -> o b"),
                              in_=ot[:])

        def emit_all(rows0=None):
            # Software-pipelined: chunk c+1's head (X transpose + S0) is
            # emitted between chunk c's layers so its S0 tanhs use mid-chunk
            # ACT slack instead of queueing behind chunk c's last H tanhs,
            # and the PE never waits on the S-update -> S0 serial chain.
            n_chunks = B // BT
            xt, s = emit_head(0, rows0)
            for c in range(n_chunks):
                emit_layer(0, xt, s)
                prev = (c, s)
                if c + 1 < n_chunks:
                    nxt = emit_head(c + 1)
                emit_layer(1, xt, s)
                emit_layer(2, xt, s)
                emit_final(*prev)
                if c + 1 < n_chunks:
                    xt, s = nxt

        if reps == 1:
            rows0 = load_x(0)
            emit_weight_dmas()
            emit_all(rows0)
        else:           # device-side repetition loop, for benchmarking only
            emit_weight_dmas()
            with tc.For_i(0, reps):
                emit_all()

    nc.compile()
    return nc


_NC = None


def _get_nc():
    global _NC
    if _NC is None:
        _NC = _build()
    return _NC


def _run(inputs, **kw):
    nc = _get_nc()
    names = (["W0", "b0"] + [f"U{g}" for g in GATES] + [f"W{g}" for g in GATES]
             + [f"b{g}" for g in GATES] + ["Wf", "bf"])
    shared = {n: np.ascontiguousarray(np.asarray(inputs[n], np.float32))
              for n in names}
    X = np.asarray(inputs["X"], np.float32)
    in_maps = [dict(shared, X=np.ascontiguousarray(X[i * B:(i + 1) * B]))
               for i in range(N_CORES)]
    res = run_bass_kernel_spmd(nc, in_maps, list(range(N_CORES)), **kw)
    out = np.concatenate([res.results[i]["out"] for i in range(N_CORES)], axis=0)
    return out, res


def kernel(**inputs) -> np.ndarray:
    out, _ = _run(inputs)
    return out


# revision 30
# speedup vs baseline: 120.9822x; 7.1758x over previous
"""DGM-net forward kernel for Trainium2, 8-core data parallel.

Network (per batch row x of width 101, n_nodes=512, 3 layers):
    S = tanh(x @ W0 + b0)
    for i in 0..2:
        Z = tanh(x @ Uz[i] + S @ Wz[i] + bz[i])
        G = tanh(x @ Ug[i] + S @ Wg[i] + bg[i])
        R = tanh(x @ Ur[i] + S @ Wr[i] + br[i])
        H = tanh(x @ Uh[i] + (S*R) @ Wh[i] + bh[i])
        S = (1-G)*H + Z*S
    out = S @ Wf + bf

Layout strategy: activations are kept feature-major (transposed:
[feature partitions, batch free]) so every matmul uses the weight matrix
in its NATURAL layout as the stationary lhsT operand and the activation
as the moving rhs: out^T[n,b] = sum_k W[k,n] S^T[k,b].  The only
transpose in the whole chain is X -> X^T once per batch chunk, done on
the PE via an identity matmul.

Batch is tiled into chunks of 512 (= one PSUM bank in fp32 and the fp32
moving-operand max).  All weights stay resident in SBUF; per chunk the
whole 3-layer network runs fused without touching DRAM except the X load
and the [1 x 512] output store.

Matmuls run as float32r (PE relaxed-precision fp32: 1 cycle/row vs plain
fp32's 4).  fp32r operands must be *produced* as fp32r, so weight DRAM
params are declared float32r (DMA passthrough) and activation producers
(tanh / DVE updates) write float32r directly.  Accumulation is fp32.
"""
import numpy as np
from contextlib import ExitStack

import concourse.bacc as bacc
import concourse.mybir as mybir
import concourse.tile as tile
from concourse.bass_utils import run_bass_kernel_spmd
from concourse.masks import make_identity

N_CORES = 8
B_FULL = 65536
B = B_FULL // N_CORES      # rows per core
D = 101                    # input width
N = 512                    # n_nodes
L = 3                      # layers
BT = 512                   # batch chunk (free dim of matmuls)
NT = N // 128              # output-feature tiles per gate
KT = N // 128              # contraction tiles for S@W
FP = mybir.dt.float32
FR = mybir.dt.float32r

GATES = ("z", "g", "r", "h")


def _build(mm_dt=FR, weight_dt=FR, reps=1):
    nc = bacc.Bacc(None)
    Tanh = mybir.ActivationFunctionType.Tanh
    WDT = weight_dt                 # stationary (lhsT) weight dtype
    ADT = mm_dt                     # moving (rhs) activation dtype

    X = nc.declare_dram_parameter("X", [B, D], FP, isOutput=False)
    W0d = nc.declare_dram_parameter("W0", [D, N], WDT, isOutput=False)
    b0d = nc.declare_dram_parameter("b0", [1, N], FP, isOutput=False)
    Ud = {g: nc.declare_dram_parameter(f"U{g}", [L, D, N], WDT, isOutput=False)
          for g in GATES}
    Wd = {g: nc.declare_dram_parameter(f"W{g}", [L, N, N], WDT, isOutput=False)
          for g in GATES}
    bd = {g: nc.declare_dram_parameter(f"b{g}", [L, 1, N], FP, isOutput=False)
          for g in GATES}
    Wfd = nc.declare_dram_parameter("Wf", [N, 1], WDT, isOutput=False)
    bfd = nc.declare_dram_parameter("bf", [1, 1], FP, isOutput=False)
    OUT = nc.declare_dram_parameter("out", [B, 1], FP, isOutput=True)

    with tile.TileContext(nc) as tc, ExitStack() as ctx:
        consts = ctx.enter_context(tc.tile_pool(name="consts", bufs=1))
        xpool = ctx.enter_context(tc.tile_pool(name="x", bufs=8))
        xtpool = ctx.enter_context(tc.tile_pool(name="xt", bufs=2))
        spool = ctx.enter_context(tc.tile_pool(name="s", bufs=2))
        zpool = ctx.enter_context(tc.tile_pool(name="z", bufs=1))
        gpool = ctx.enter_context(tc.tile_pool(name="g", bufs=1))
        rpool = ctx.enter_context(tc.tile_pool(name="r", bufs=1))
        hpool = ctx.enter_context(tc.tile_pool(name="h", bufs=1))
        opool = ctx.enter_context(tc.tile_pool(name="o", bufs=2))
        psum = ctx.enter_context(tc.tile_pool(name="psum", bufs=5, space="PSUM"))
        psum_t = ctx.enter_context(tc.tile_pool(name="psum_t", bufs=2, space="PSUM"))
        psum_f = ctx.enter_context(tc.tile_pool(name="psum_f", bufs=1, space="PSUM"))

        ident = consts.tile([128, 128], FP)
        make_identity(nc, ident[:])

        # --- resident weights, all in natural (k-major) layout ---
        # Weight DMAs go through GpSimd's SWDGE queue: its sequencer has no
        # compute role, so weight streaming never blocks the ACT/SP
        # instruction streams (X loads / tanh).  Issued per layer in the
        # order the first chunk consumes them so the PE starts ASAP.
        def wdma(out, in_):
            nc.gpsimd.dma_start(out=out, in_=in_)

        # per-(gate,layer) tiles so dependency tracking and DMA arrival are
        # layer-granular: layer-0 matmuls start as soon as layer-0 weights land
        w0 = consts.tile([D, N], WDT)
        b0t = consts.tile([128, NT], FP)
        u, w, bias = {}, {}, {}
        for g in GATES:
            for l in range(L):
                u[g, l] = consts.tile([D, N], WDT, name=f"u_{g}{l}")
                w[g, l] = consts.tile([128, KT, N], WDT, name=f"w_{g}{l}")
            bias[g] = consts.tile([128, L * NT], FP, name=f"bias_{g}")
        wf = consts.tile([128, KT], WDT)
        bft = consts.tile([1, 1], FP)

        def emit_weight_dmas():
            wdma(w0[:], W0d[:])
            wdma(b0t[:], b0d[:].rearrange("o (nt p) -> p (o nt)", p=128))
            for g in GATES:
                wdma(bias[g][:], bd[g][:].rearrange("l o (nt p) -> p (l o nt)", p=128))
            for l in range(L):
                for g in GATES:
                    wdma(u[g, l][:], Ud[g][l].rearrange("p n -> p n"))
                    # W split in half so both queues stream each matrix
                    wdma(w[g, l][:, 0:2],
                         Wd[g][l, 0:256].rearrange("(kt p) n -> p kt n", p=128))
                    wdma(w[g, l][:, 2:4],
                         Wd[g][l, 256:512].rearrange("(kt p) n -> p kt n", p=128))
            wdma(wf[:], Wfd[:].rearrange("(kt p) o -> p (kt o)", p=128))
            wdma(bft[:], bfd[:])

        sub, mult = mybir.AluOpType.subtract, mybir.AluOpType.mult

        def f32(ap):            # read fp32r bits as plain fp32 (DVE/ACT reads)
            return ap.bitcast(FP) if ap.dtype == FR else ap

        def load_x(c):
            r0 = c * BT
            rows = []
            for i in range(BT // 128):
                xrow = xpool.tile([128, D], FP, name="xrow")
                nc.sync.dma_start(out=xrow[:],
                                  in_=X[r0 + i * 128: r0 + (i + 1) * 128, :])
                rows.append(xrow)
            return rows

        def emit_head(c, rows=None):
            """X load + PE transpose + S0 for chunk c; returns (xt, s)."""
            if rows is None:
                rows = load_x(c)
            xt = xtpool.tile([D, BT], ADT)
            tp = psum_t.tile([D, BT], FP)      # one PSUM bank, 4 transposes
            for i in range(BT // 128):
                nc.tensor.transpose(tp[:, i * 128:(i + 1) * 128], rows[i][:],
                                    ident[:])
            nc.scalar.copy(xt[:], tp[:])

            # ---- S0 = tanh(X @ W0 + b0) ----
            s = spool.tile([128, KT, BT], ADT)
            for nt in range(NT):
                acc = psum.tile([128, BT], FP)
                nc.tensor.matmul(acc[:], w0[:, nt * 128:(nt + 1) * 128],
                                 xt[:], start=True, stop=True)
                nc.scalar.activation(s[:, nt, :], acc[:], Tanh,
                                     bias=b0t[:, nt:nt + 1])
            return xt, s

        def emit_layer(l, xt, s):
            if True:
                zt = zpool.tile([128, NT, BT], FP)
                gt = gpool.tile([128, NT, BT], FP)
                rt = rpool.tile([128, NT, BT], ADT)
                ht = hpool.tile([128, NT, BT], FP)
                # Z, G, R gates: tanh(X@U + S@W + b)
                for g, dest in (("z", zt), ("g", gt), ("r", rt)):
                    for nt in range(NT):
                        acc = psum.tile([128, BT], FP)
                        nc.tensor.matmul(
                            acc[:], u[g, l][:, nt * 128:(nt + 1) * 128],
                            xt[:], start=True, stop=False)
                        for kt in range(KT):
                            nc.tensor.matmul(
                                acc[:],
                                w[g, l][:, kt, nt * 128:(nt + 1) * 128],
                                s[:, kt, :], start=False, stop=(kt == KT - 1))
                        nc.scalar.activation(dest[:, nt, :], acc[:], Tanh,
                                             bias=bias[g][:, l * NT + nt:l * NT + nt + 1])
                # R <- S*R (in place, rounded to fp32r; R only feeds (S*R)@Wh)
                for kt in range(KT):
                    nc.vector.tensor_mul(rt[:, kt, :], f32(s[:, kt, :]),
                                         f32(rt[:, kt, :]))
                # H = tanh(X@Uh + (S*R)@Wh + bh)
                for nt in range(NT):
                    acc = psum.tile([128, BT], FP)
                    nc.tensor.matmul(
                        acc[:], u["h", l][:, nt * 128:(nt + 1) * 128],
                        xt[:], start=True, stop=False)
                    for kt in range(KT):
                        nc.tensor.matmul(
                            acc[:], w["h", l][:, kt, nt * 128:(nt + 1) * 128],
                            rt[:, kt, :], start=False, stop=(kt == KT - 1))
                    nc.scalar.activation(ht[:, nt, :], acc[:], Tanh,
                                         bias=bias["h"][:, l * NT + nt:l * NT + nt + 1])
                # S = Z*S + (1-G)*H   (3 DVE ops per feature tile, in place)
                for nt in range(NT):
                    nc.vector.tensor_mul(zt[:, nt, :], zt[:, nt, :],
                                         f32(s[:, nt, :]))
                    nc.vector.scalar_tensor_tensor(
                        gt[:, nt, :], gt[:, nt, :], 1.0, ht[:, nt, :],
                        op0=sub, op1=mult)          # (G-1)*H
                    nc.vector.tensor_sub(s[:, nt, :], zt[:, nt, :], gt[:, nt, :])

        def emit_final(c, s):
            # ---- out = S @ Wf + bf ----
            r0 = c * BT
            accf = psum_f.tile([1, BT], FP)
            for kt in range(KT):
                nc.tensor.matmul(accf[:], wf[:, kt:kt + 1], s[:, kt, :],
                                 start=(kt == 0), stop=(kt == KT - 1))
            ot = opool.tile([1, BT], FP)
            nc.vector.tensor_scalar_add(ot[:], accf[:], bft[0:1, 0:1])
            nc.sync.dma_start(out=OUT[r0:r0 + BT, 0:1].rearrange("b o -> o b"),
                              in_=ot[:])

        def emit_all(rows0=None):
            # Software-pipelined: chunk c+1's head (X transpose + S0) is
            # emitted between chunk c's layers so its S0 tanhs use mid-chunk
            # ACT slack instead of queueing behind chunk c's last H tanhs,
            # and the PE never waits on the S-update -> S0 serial chain.
            n_chunks = B // BT
            xt, s = emit_head(0, rows0)
            for c in range(n_chunks):
                emit_layer(0, xt, s)
                prev = (c, s)
                if c + 1 < n_chunks:
                    nxt = emit_head(c + 1)
                emit_layer(1, xt, s)
                emit_layer(2, xt, s)
                emit_final(*prev)
                if c + 1 < n_chunks:
                    xt, s = nxt

        if reps == 1:
            rows0 = load_x(0)
            emit_weight_dmas()
            emit_all(rows0)
        else:           # device-side repetition loop, for benchmarking only
            emit_weight_dmas()
            with tc.For_i(0, reps):
                emit_all()

    nc.compile()
    return nc


_NC = None


def _get_nc():
    global _NC
    if _NC is None:
        _NC = _build()
    return _NC


WEIGHT_NAMES = ["W0"] + [f"U{g}" for g in GATES] + [f"W{g}" for g in GATES] + ["Wf"]
BIAS_NAMES = ["b0"] + [f"b{g}" for g in GATES] + ["bf"]


def prep_shared(inputs):
    return {n: np.ascontiguousarray(np.asarray(inputs[n], np.float32))
            for n in WEIGHT_NAMES + BIAS_NAMES}


def _run(inputs, **kw):
    nc = _get_nc()
    shared = prep_shared(inputs)
    X = np.asarray(inputs["X"], np.float32)
    in_maps = [dict(shared, X=np.ascontiguousarray(X[i * B:(i + 1) * B]))
               for i in range(N_CORES)]
    res = run_bass_kernel_spmd(nc, in_maps, list(range(N_CORES)), **kw)
    out = np.concatenate([res.results[i]["out"] for i in range(N_CORES)], axis=0)
    return out, res


def kernel(**inputs) -> np.ndarray:
    out, _ = _run(inputs)
    return out
